# revision 4
# baseline (speedup 1.0000x reference)
"""MoE feed-forward (8 experts, top-2, D=1024, H=4096) on 8 Trainium2 cores.

Strategy: expert-parallel with host-side routing.
  - Host computes the gating (logits -> top-2 -> softmax) in fp64 and
    gathers each expert's tokens into a padded, transposed activation
    matrix xT [D, C] (C = padded per-expert token capacity).
  - Core e runs the dense FFN for expert e only over its routed tokens:
        y = gelu(x @ w1[e] + b1[e]) @ w2[e]
    The hidden dim is processed in 4 quarters of 1024; each quarter's
    weights (8 MB fp32) are resident in SBUF and double-buffered, so the
    next quarter's weights stream in during the current quarter's
    compute. Each quarter emits a partial output yp[q] contracted over
    its hidden slice; matmuls use float32r (~1.5e-4 rel err) and the
    exact-erf Gelu of the scalar engine.
  - Host combines: out[tok] += p_e * (sum_q yp[q] + b2[e]).

Self-contained: hardcodes all shapes from the problem spec.
"""

import numpy as np

import concourse.bass as bass
import concourse.mybir as mybir
import concourse.tile as tile
from concourse.bass_utils import run_bass_kernel_spmd

F32 = mybir.dt.float32
F32R = mybir.dt.float32r

D_MODEL = 1024
HIDDEN = 4096
N_EXPERTS = 8
TOP_K = 2
NQ = 4                    # hidden-dim quarters
HQ = HIDDEN // NQ         # 1024 hidden units per quarter
DBLK = D_MODEL // 128     # 8
JQ = HQ // 128            # 8
CT = 384                  # token tile (matmul moving free dim, 3*128)


# ---------------------------------------------------------------------------
# Walrus workaround: this container's Tile emits instructions carrying more
# sync waits than the bundled walrus accepts ("Too many sync wait commands").
# Hoist excess waits onto EventSemaphore instructions placed immediately
# before the overloaded instruction (same engine, same block) — semantically
# identical: the engine blocks on each wait in program order.
_CAP_BY_OPCODE = {"EventSemaphore": 2}
_DEFAULT_CAP = 1
_split_counter = [0]


def split_excess_waits(nc):
    for f in nc.m.functions:
        for bb in f.blocks:
            new_insts = []
            changed = False
            for inst in bb.instructions:
                si = inst.sync_info
                waits = list(si.on_wait) if si is not None else []
                cap = _CAP_BY_OPCODE.get(inst.opcode, _DEFAULT_CAP)
                if len(waits) > cap:
                    changed = True
                    excess, keep = waits[:-cap], waits[-cap:]
                    for i in range(0, len(excess), 2):
                        _split_counter[0] += 1
                        new_insts.append(mybir.InstEventSemaphore(
                            name=f"I-waitsplit-{_split_counter[0]}",
                            engine=inst.engine,
                            sync_info=mybir.SyncInfo(
                                on_wait=excess[i:i + 2], on_update=[]),
                        ))
                    inst.sync_info = mybir.SyncInfo(
                        on_wait=keep, on_update=list(si.on_update))
                new_insts.append(inst)
            if changed:
                bb.instructions = new_insts
    return nc


# ---------------------------------------------------------------------------
def build_nc(C, act=None, reps=1, bufs_x=3, bufs_h=2, bufs_ps1=2, bufs_ps2=2, bufs_y=3):
    """Per-core FFN program: xT [D, C] -> yp [NQ, C, D] (partial per
    hidden-quarter; host sums the quarters)."""
    if act is None:
        act = mybir.ActivationFunctionType.Gelu
    assert C % CT == 0
    NCT = C // CT
    nc = bass.Bass()
    xT = nc.dram_tensor("xT", [D_MODEL, C], F32R, kind="ExternalInput")
    w1 = nc.dram_tensor("w1", [D_MODEL, HIDDEN], F32R, kind="ExternalInput")
    b1 = nc.dram_tensor("b1", [HIDDEN], F32, kind="ExternalInput")
    w2 = nc.dram_tensor("w2", [HIDDEN, D_MODEL], F32R, kind="ExternalInput")
    yp = nc.dram_tensor("yp", [NQ, C, D_MODEL], F32, kind="ExternalOutput")

    with tile.TileContext(nc) as tc:
        with (
            tc.tile_pool(name="wpool", bufs=2) as wpool,
            tc.tile_pool(name="xpool", bufs=bufs_x) as xpool,
            tc.tile_pool(name="hpool", bufs=bufs_h) as hpool,
            tc.tile_pool(name="ypool", bufs=bufs_y) as ypool,
            tc.tile_pool(name="ps1", bufs=bufs_ps1, space="PSUM") as ps1,
            tc.tile_pool(name="ps2", bufs=bufs_ps2, space="PSUM") as ps2,
        ):
            def whole(_=None):
                for q in range(NQ):
                    # this quarter's weights (double-buffered across q)
                    w1t = wpool.tile([128, DBLK, HQ], F32R, tag="w1t")
                    for d in range(DBLK):
                        nc.sync.dma_start(
                            out=w1t[:, d, :],
                            in_=w1.ap()[d * 128:(d + 1) * 128,
                                        q * HQ:(q + 1) * HQ])
                    w2t = wpool.tile([128, JQ, D_MODEL], F32R, tag="w2t")
                    for j in range(JQ):
                        nc.sync.dma_start(
                            out=w2t[:, j, :],
                            in_=w2.ap()[q * HQ + j * 128:
                                        q * HQ + (j + 1) * 128, :])
                    b1t = wpool.tile([128, JQ], F32, tag="b1t")
                    nc.sync.dma_start(
                        out=b1t[:],
                        in_=b1.ap()[q * HQ:(q + 1) * HQ]
                        .rearrange("(b p) -> p b", p=128))

                    for ct in range(NCT):
                        xt = xpool.tile([128, DBLK, CT], F32R, tag="xt")
                        nc.sync.dma_start(
                            out=xt[:],
                            in_=xT.ap()[:, ct * CT:(ct + 1) * CT]
                            .rearrange("(b p) c -> p b c", p=128))

                        hT = hpool.tile([128, JQ * CT], F32R, tag="hT")
                        for j in range(JQ):
                            ps = ps1.tile([128, CT], F32, tag="ps")
                            for d in range(DBLK):
                                nc.tensor.matmul(
                                    ps[:],
                                    w1t[:, d, j * 128:(j + 1) * 128],
                                    xt[:, d, :],
                                    start=(d == 0), stop=(d == DBLK - 1))
                            nc.scalar.activation(
                                hT[:, j * CT:(j + 1) * CT], ps[:], act,
                                bias=b1t[:, j:j + 1])

                        for cs in range(CT // 128):
                            for dh in range(2):
                                p2 = ps2.tile([128, 512], F32, tag="p2")
                                for j in range(JQ):
                                    nc.tensor.matmul(
                                        p2[:],
                                        hT[:, j * CT + cs * 128:
                                           j * CT + (cs + 1) * 128],
                                        w2t[:, j, dh * 512:(dh + 1) * 512],
                                        start=(j == 0), stop=(j == JQ - 1))
                                yb = ypool.tile([128, 512], F32, tag="yb")
                                nc.vector.tensor_copy(yb[:], p2[:])
                                nc.sync.dma_start(
                                    out=yp.ap()[q,
                                                ct * CT + cs * 128:
                                                ct * CT + (cs + 1) * 128,
                                                dh * 512:(dh + 1) * 512],
                                    in_=yb[:])

            if reps == 1:
                whole()
            else:
                with tc.For_i(0, reps, 1):
                    whole()
    return nc


# ---------------------------------------------------------------------------
def _gating(x2d, gate_w, gate_b):
    """fp64 host gating; returns per-expert (idx, prob) matching jax top_k
    (ties -> lower index, measure-zero for random inputs)."""
    logits = x2d.astype(np.float64) @ gate_w.astype(np.float64) \
        + gate_b.astype(np.float64)
    i1 = np.argmax(logits, axis=-1)
    n = len(logits)
    ar = np.arange(n)
    v1 = logits[ar, i1]
    l2 = logits.copy()
    l2[ar, i1] = -np.inf
    i2 = np.argmax(l2, axis=-1)
    v2 = l2[ar, i2]
    m = np.maximum(v1, v2)
    e1 = np.exp(v1 - m)
    e2 = np.exp(v2 - m)
    s = e1 + e2
    p1 = (e1 / s)
    p2 = (e2 / s)
    out = []
    for e in range(N_EXPERTS):
        m1 = i1 == e
        m2 = i2 == e
        idx = np.nonzero(m1 | m2)[0]
        prob = np.where(m1, p1, p2)[idx].astype(np.float32)
        out.append((idx, prob))
    return out


_NC_CACHE = {}


def kernel(x, gate_w, gate_b, w1, b1, w2, b2):
    x = np.asarray(x, dtype=np.float32)
    gate_w = np.asarray(gate_w, dtype=np.float32)
    gate_b = np.asarray(gate_b, dtype=np.float32)
    w1 = np.asarray(w1, dtype=np.float32)
    b1 = np.asarray(b1, dtype=np.float32)
    w2 = np.asarray(w2, dtype=np.float32)
    b2 = np.asarray(b2, dtype=np.float32)

    B, T, D = x.shape
    x2d = x.reshape(-1, D)
    routes = _gating(x2d, gate_w, gate_b)

    max_n = max(len(idx) for idx, _ in routes)
    C = max(6 * CT, -(-max_n // CT) * CT)

    if C not in _NC_CACHE:
        nc = build_nc(C)
        split_excess_waits(nc)
        _NC_CACHE[C] = nc
    nc = _NC_CACHE[C]

    in_maps = []
    for e in range(N_EXPERTS):
        idx, _ = routes[e]
        xTe = np.zeros((D_MODEL, C), dtype=np.float32)
        xTe[:, :len(idx)] = x2d[idx].T
        in_maps.append({
            "xT": xTe,
            "w1": np.ascontiguousarray(w1[e]),
            "b1": np.ascontiguousarray(b1[e]),
            "w2": np.ascontiguousarray(w2[e]),
        })

    res = run_bass_kernel_spmd(nc, in_maps, core_ids=list(range(N_EXPERTS)))

    out2d = np.zeros((B * T, D_MODEL), dtype=np.float32)
    for e in range(N_EXPERTS):
        idx, prob = routes[e]
        ypart = res.results[e]["yp"]
        n = len(idx)
        y_e = ypart[0, :n] + ypart[1, :n] + ypart[2, :n] + ypart[3, :n] + b2[e]
        out2d[idx] += prob[:, None] * y_e
    return out2d.reshape(B, T, D_MODEL)
